# revision 32
# baseline (speedup 1.0000x reference)
"""CRF loss (forward-algorithm log-partition minus gold-path score) on 8 trn2 cores.

Strategy (data-parallel over B, 32 rows per core, 2 interleaved scan chains):
  Denominator: forward scan in probability space with E = exp(transitions)
  as the bf16 PE stationary. The 32 rows are split into two chains of 16
  (cols 0-15 / 16-31) that advance independently so one chain's bf16 matmul
  overlaps the other's PSUM->SBUF multiply (both multiplies on the DVE;
  GPSIMD cannot read PSUM on TRN2). Emissions arrive from the host already
  exponentiated (exp(em - SHIFT), bf16); SHIFT = log(128)+0.5 cancels the
  expected per-step log-growth so everything stays in range.
  Masking: lengths are >= 256, so log(row-sum) is snapshotted for every
  t in [240, 511] (17 chunks of 16 steps, rowsum by ones-matmul + Ln) and
  a host-built 0/1 select vector picks t = len_b - 1 per row via a
  GPSIMD multiply + DVE reduce per chunk, all in SBUF (no DRAM roundtrip);
  the host adds SHIFT * sum(len).
  Numerator: host ships masked one-hot tag encodings; emission gather and
  transition-pair counts accumulate in PSUM via one matmul per 2 scan
  steps, with per-row tiles prefetched a full window ahead so the in-order
  PE queue never head-of-line blocks the scan.
Output per core: scalar sum over its rows of (log_den_unshifted - log_num);
host adds the SHIFT correction and divides by B.
"""

import numpy as np
import ml_dtypes

B, T, C = 256, 512, 128
NCORES = 8
BL = B // NCORES
HB = BL // 2                      # rows per scan chain
SHIFT = float(np.log(128.0) + 0.5)
SNAP_T0 = 240                     # first snapshotted t (lengths >= 256)
NSNAP = (T - SNAP_T0) // 16       # 17 chunks of 16 steps
NCH = T // 128                    # 4 numerator t-chunks per row

_cache = {}


def _build_program():
    import concourse.bass as bass
    import concourse.bacc as bacc
    import concourse.tile as tile
    from concourse import mybir

    f32 = mybir.dt.float32
    bf16 = mybir.dt.bfloat16
    Alu = mybir.AluOpType
    Act = mybir.ActivationFunctionType
    Axis = mybir.AxisListType

    nc = bacc.Bacc(None)

    expem_in = nc.dram_tensor("expem_ctb", [C, T, BL], bf16, kind="ExternalInput")
    em_btc = nc.dram_tensor("em_btc", [BL, T, C], bf16, kind="ExternalInput")
    oh_btc = nc.dram_tensor("oh_btc", [BL, T, C], bf16, kind="ExternalInput")
    ohs_btc = nc.dram_tensor("ohs_btc", [BL, T, C], bf16, kind="ExternalInput")
    selw_in = nc.dram_tensor("selw", [NSNAP * 512], f32, kind="ExternalInput")
    trans_in = nc.dram_tensor("trans", [C, C], f32, kind="ExternalInput")
    ebf_in = nc.dram_tensor("ebf", [C, C], bf16, kind="ExternalInput")
    out_d = nc.dram_tensor("out", [1, 1], f32, kind="ExternalOutput")
    dbg_emit = nc.dram_tensor("dbg_emit", [C, 1], f32, kind="ExternalOutput")
    dbg_pair = nc.dram_tensor("dbg_pair", [C, 1], f32, kind="ExternalOutput")
    dbg_den = nc.dram_tensor("dbg_den", [1, NSNAP], f32, kind="ExternalOutput")

    ident_in = nc.inline_tensor(np.eye(C, dtype=np.float32), name="ident")
    onesb_in = nc.inline_tensor(
        np.ones((C, 1), ml_dtypes.bfloat16), name="onesb"
    )
    onesf_in = nc.inline_tensor(np.ones((C, 1), np.float32), name="onesf")

    with tile.TileContext(nc) as tc:
        with (
            tc.tile_pool(name="consts", bufs=1) as consts,
            tc.tile_pool(name="bigbuf", bufs=1) as bigbuf,
            tc.tile_pool(name="psu", bufs=2, space="PSUM") as psu,
            tc.tile_pool(name="accps", bufs=1, space="PSUM") as accps,
            tc.tile_pool(name="rsps", bufs=1, space="PSUM") as rsps,
            tc.tile_pool(name="oh", bufs=3) as ohpool,
            tc.tile_pool(name="ohs", bufs=3) as ohspool,
            tc.tile_pool(name="emn", bufs=4) as emnpool,
            tc.tile_pool(name="logc", bufs=2) as logcpool,
        ):
            # ---------- constants / small inputs ----------
            ebf_sb = consts.tile([C, C], bf16)
            nc.sync.dma_start(out=ebf_sb[:], in_=ebf_in[:])
            trans_sb = consts.tile([C, C], f32)
            nc.sync.dma_start(out=trans_sb[:], in_=trans_in[:])
            ident_sb = consts.tile([C, C], f32)
            nc.sync.dma_start(out=ident_sb[:], in_=ident_in[:])
            onesb_sb = consts.tile([C, 1], bf16)
            nc.sync.dma_start(out=onesb_sb[:], in_=onesb_in[:])
            onesf_sb = consts.tile([C, 1], f32)
            nc.sync.dma_start(out=onesf_sb[:], in_=onesf_in[:])
            selflat = consts.tile([1, NSNAP * 512], f32)
            nc.sync.dma_start(out=selflat[:], in_=selw_in[:])
            den_part = consts.tile([1, NSNAP], f32)
            junk = consts.tile([1, 512], f32)

            # ---------- big buffers ----------
            expem = bigbuf.tile([C, T, BL], bf16)
            ECH = 64
            for k in range(T // ECH):
                nc.sync.dma_start(
                    out=expem[:, k * ECH : (k + 1) * ECH, :],
                    in_=expem_in[:, k * ECH : (k + 1) * ECH, :],
                )
            CH_N = [16, 16]                # rows per chain
            CH_O = [0, 16]                 # row offsets
            S_bufs = [
                bigbuf.tile([C, T, n], bf16, name=f"S{g}")
                for g, n in enumerate(CH_N)
            ]

            emit_ps = accps.tile([C, C], f32)
            pair_ps = accps.tile([C, C], f32)

            num_tiles = {}
            em_t0 = emnpool.tile([128, NCH, C], bf16, tag="em")
            nc.sync.dma_start(
                out=em_t0[:], in_=em_btc[0].rearrange("(h l) c -> l h c", l=128)
            )
            oh0 = ohpool.tile([128, NCH, C], bf16, tag="oh")
            nc.sync.dma_start(
                out=oh0[:], in_=oh_btc[0].rearrange("(h l) c -> l h c", l=128)
            )
            ohs0 = ohspool.tile([128, NCH, C], bf16, tag="ohs")
            nc.sync.dma_start(
                out=ohs0[:], in_=ohs_btc[0].rearrange("(h l) c -> l h c", l=128)
            )
            num_tiles[0] = (em_t0, oh0, ohs0)

            # ---------- fused scan + numerator + snapshots ----------
            NG = 2
            for t in range(1, T):
                us = {}
                for g in range(NG):
                    o, n = CH_O[g], CH_N[g]
                    rhs = (
                        expem[:, 0, o : o + n] if t == 1
                        else S_bufs[g][:, t - 1, :]
                    )
                    u = psu.tile([C, n], f32, tag=f"u{g}")
                    nc.tensor.matmul(
                        u[:], lhsT=ebf_sb[:], rhs=rhs, start=True, stop=True
                    )
                    us[g] = u
                for g in range(NG):
                    o, n = CH_O[g], CH_N[g]
                    nc.vector.tensor_tensor(
                        out=S_bufs[g][:, t, :], in0=us[g][:],
                        in1=expem[:, t, o : o + n], op=Alu.mult,
                    )

                # numerator tile prefetch: row b's 3 tensors in 3 DMAs,
                # fetched one 16-step window before first use
                if (t - 2) % 16 == 0 and (t - 2) // 16 + 1 < BL:
                    bf_row = (t - 2) // 16 + 1
                    em_t = emnpool.tile([128, NCH, C], bf16, tag="em")
                    nc.sync.dma_start(
                        out=em_t[:],
                        in_=em_btc[bf_row].rearrange("(h l) c -> l h c", l=128),
                    )
                    oh = ohpool.tile([128, NCH, C], bf16, tag="oh")
                    nc.sync.dma_start(
                        out=oh[:],
                        in_=oh_btc[bf_row].rearrange("(h l) c -> l h c", l=128),
                    )
                    ohs = ohspool.tile([128, NCH, C], bf16, tag="ohs")
                    nc.sync.dma_start(
                        out=ohs[:],
                        in_=ohs_btc[bf_row].rearrange("(h l) c -> l h c", l=128),
                    )
                    num_tiles[bf_row % 3] = (em_t, oh, ohs)

                # numerator matmuls: one matmul per 2 steps (emit and
                # pair alternate) so each fits the PE's idle window
                if t >= 2 and (t - 2) // 2 < 2 * BL * NCH:
                    j = (t - 2) // 2 if t % 2 == 0 else None
                    if j is not None:
                        i, which = j // 2, j % 2
                        b, ch = i // NCH, i % NCH
                        em_t, oh, ohs = num_tiles[b % 3]
                        if which == 0:
                            nc.tensor.matmul(
                                emit_ps[:], lhsT=oh[:, ch, :],
                                rhs=em_t[:, ch, :],
                                start=(i == 0), stop=(i == BL * NCH - 1),
                                skip_group_check=True,
                            )
                        else:
                            nc.tensor.matmul(
                                pair_ps[:], lhsT=oh[:, ch, :],
                                rhs=ohs[:, ch, :],
                                start=(i == 0), stop=(i == BL * NCH - 1),
                                skip_group_check=True,
                            )

                # snapshot chunk k: log row-sums for t in [240+16k, 255+16k],
                # emitted 2 steps after the chunk completes so the PE never
                # head-of-line blocks on a same-step dependency
                if (
                    t >= SNAP_T0 + 17
                    and (t - SNAP_T0 - 17) % 16 == 0
                    and (t - SNAP_T0 - 17) // 16 < NSNAP - 1
                ):
                    k = (t - SNAP_T0 - 17) // 16
                    ts = SNAP_T0 + 16 * k
                    rs = rsps.tile([1, 16 * BL], f32, tag="rs")
                    off = 0
                    for g in range(len(CH_N)):
                        n16 = 16 * CH_N[g]
                        nc.tensor.matmul(
                            rs[:, off : off + n16], lhsT=onesb_sb[:, :1],
                            rhs=S_bufs[g][:, ts : ts + 16, :],
                            start=True, stop=True,
                        )
                        off += n16
                    logc = logcpool.tile([1, 2 * 16 * HB], f32, tag="logc")
                    nc.scalar.activation(out=logc[:], in_=rs[:], func=Act.Ln)
                    nc.gpsimd.tensor_tensor(
                        out=junk[:], in0=logc[:],
                        in1=selflat[:, 512 * k : 512 * (k + 1)],
                        op=Alu.mult,
                    )
                    nc.vector.tensor_reduce(
                        out=den_part[:, k : k + 1], in_=junk[:],
                        axis=Axis.X, op=Alu.add,
                    )

            # last numerator unit (slot 255 doesn't fit the in-loop
            # cadence): the pair matmul that closes the accumulation group
            em_t, oh, ohs = num_tiles[(BL - 1) % 3]
            nc.tensor.matmul(
                pair_ps[:], lhsT=oh[:, NCH - 1, :], rhs=ohs[:, NCH - 1, :],
                start=False, stop=True, skip_group_check=True,
            )

            # final snapshot chunk (t in [496, 511])
            k = NSNAP - 1
            ts = SNAP_T0 + 16 * k
            rs = rsps.tile([1, 16 * BL], f32, tag="rs")
            off = 0
            for g in range(len(CH_N)):
                n16 = 16 * CH_N[g]
                nc.tensor.matmul(
                    rs[:, off : off + n16], lhsT=onesb_sb[:, :1],
                    rhs=S_bufs[g][:, ts : ts + 16, :],
                    start=True, stop=True,
                )
                off += n16
            logc = logcpool.tile([1, 16 * BL], f32, tag="logc")
            nc.scalar.activation(out=logc[:], in_=rs[:], func=Act.Ln)
            nc.gpsimd.tensor_tensor(
                out=junk[:], in0=logc[:],
                in1=selflat[:, 512 * k : 512 * (k + 1)], op=Alu.mult,
            )
            nc.vector.tensor_reduce(
                out=den_part[:, k : k + 1], in_=junk[:],
                axis=Axis.X, op=Alu.add,
            )

            # ---------- epilogue ----------
            den_tot = consts.tile([1, 1], f32)
            nc.vector.tensor_reduce(
                out=den_tot[:], in_=den_part[:], axis=Axis.X, op=Alu.add
            )

            emit_acc = consts.tile([C, 1], f32)
            nc.vector.tensor_tensor(
                out=emit_ps[:], in0=emit_ps[:], in1=ident_sb[:], op=Alu.mult
            )
            nc.vector.tensor_reduce(
                out=emit_acc[:], in_=emit_ps[:], axis=Axis.X, op=Alu.add
            )
            pair_acc = consts.tile([C, 1], f32)
            nc.vector.tensor_tensor(
                out=pair_ps[:], in0=pair_ps[:], in1=trans_sb[:], op=Alu.mult
            )
            nc.vector.tensor_reduce(
                out=pair_acc[:], in_=pair_ps[:], axis=Axis.X, op=Alu.add
            )

            fin = consts.tile([C, 1], f32)
            nc.vector.tensor_tensor(
                out=fin[:], in0=emit_acc[:], in1=pair_acc[:], op=Alu.add
            )
            nc.sync.dma_start(out=dbg_emit[:], in_=emit_acc[:])
            nc.sync.dma_start(out=dbg_pair[:], in_=pair_acc[:])
            nc.sync.dma_start(out=dbg_den[:], in_=den_part[:])
            fin_ps = rsps.tile([1, 1], f32, tag="fin")
            nc.tensor.matmul(
                fin_ps[:], lhsT=onesf_sb[:, :1], rhs=fin[:], start=True, stop=True
            )
            sumep = consts.tile([1, 1], f32)
            nc.scalar.copy(out=sumep[:], in_=fin_ps[:])
            res_sb = consts.tile([1, 1], f32)
            nc.vector.tensor_tensor(
                out=res_sb[:], in0=den_tot[:], in1=sumep[:], op=Alu.subtract
            )
            nc.sync.dma_start(out=out_d[:], in_=res_sb[:])

    nc.compile()
    return nc


def _prep_inputs(emissions, tags, mask, transitions):
    bf = ml_dtypes.bfloat16
    em = np.asarray(emissions, dtype=np.float32)
    tg = np.asarray(tags).astype(np.int32)
    mk = np.asarray(mask).astype(bool)
    tr = np.ascontiguousarray(np.asarray(transitions), dtype=np.float32)
    ebf = np.exp(tr).astype(bf)

    # host-side select weights: 1 at t = len_b - 1 (always in [240, 511])
    lengths = mk.sum(axis=1).astype(np.int64)  # [B]

    # masked one-hot encodings of tags and next-tags (inputs to the
    # numerator's gather-by-matmul); pure tags/mask preprocessing
    eye = np.eye(C, dtype=bf)
    oh_all = eye[tg] * mk[:, :, None].astype(bf)          # [B,T,C]
    ohs_all = np.zeros_like(oh_all)
    ohs_all[:, :-1] = oh_all[:, 1:]

    in_maps = []
    for core in range(NCORES):
        b0, b1 = core * BL, (core + 1) * BL
        em_c = em[b0:b1]

        # snapshot stream layout:
        # flat = k*512 + chain_off*16 + dt*chain_n + chain_row
        CH_N = [16, 16]
        CH_O = [0, 16]
        selw = np.zeros(NSNAP * 512, np.float32)
        for b in range(BL):
            tsel = int(lengths[b0 + b]) - 1
            assert SNAP_T0 <= tsel < T, f"length out of snapshot range: {tsel+1}"
            k, dt = divmod(tsel - SNAP_T0, 16)
            g = 0 if b < 16 else 1
            r = b - CH_O[g]
            selw[k * 512 + CH_O[g] * 16 + dt * CH_N[g] + r] = 1.0

        expem = np.exp(em_c.transpose(2, 1, 0) - SHIFT).astype(bf)  # [C,T,BL]

        in_maps.append({
            "expem_ctb": np.ascontiguousarray(expem),
            "em_btc": np.ascontiguousarray(em_c.astype(bf)),
            "oh_btc": np.ascontiguousarray(oh_all[b0:b1]),
            "ohs_btc": np.ascontiguousarray(ohs_all[b0:b1]),
            "selw": selw,
            "trans": tr,
            "ebf": ebf,
        })
    shift_corr = SHIFT * float(lengths.sum())
    return in_maps, shift_corr


def kernel(emissions, tags, mask, transitions, _want_results=False, **_run_kw):
    from concourse.bass_utils import run_bass_kernel_spmd

    if "nc" not in _cache:
        _cache["nc"] = _build_program()
    nc = _cache["nc"]

    in_maps, shift_corr = _prep_inputs(emissions, tags, mask, transitions)
    res = run_bass_kernel_spmd(nc, in_maps, core_ids=list(range(NCORES)), **_run_kw)
    total = sum(float(r["out"][0, 0]) for r in res.results) + shift_corr
    out = np.float32(total / B)
    if _want_results:
        return out, res
    return out
